# revision 13
# baseline (speedup 1.0000x reference)
"""Trainium2 Bass kernel for nn_EngramPt (key-gated value + dilated causal conv).

Strategy (8 cores, SPMD): shard tokens as (batch b, T-half) -> 8 shards of 2048
tokens. All device compute is channel-major ([C or H on partitions, tokens on
free dim]); host prep does the layout transposes / bf16 casts (sharding prep),
device does all the math, host re-assembles the channel-major shard outputs.

Device pipeline per core:
  key^T = WkT.T @ embT   (PE, bf16)        v^T = WvT.T @ embT  (PE)
  gate reductions (sum over H) via ones-matmuls on PE; squares on ACT;
  key*hs product on DVE.  Gate math on tiny [4, T] rows.
  x_norm^T = alpha_bcast * v^T ; 4-tap dilated conv on DVE; silu on ACT;
  out^T = gate_bcast * v^T + silu(conv)  -> bf16, host transposes back.

w1/w2 are folded into the key copy (ACT scale), wn into conv weights (host),
so non-ones norm weights are still handled exactly.
"""

import sys

if "/opt/trn_rl_repo" not in sys.path:
    sys.path.insert(0, "/opt/trn_rl_repo")

import numpy as np
import ml_dtypes

import concourse.bass as bass
import concourse.mybir as mybir
from concourse import bacc
from concourse.tile import TileContext
from concourse.bass_utils import run_bass_kernel_spmd

BF16 = np.float16

B, T, E, H, G = 4, 4096, 1024, 1024, 4
C = G * H                      # 4096 channels
NCORES = 8
THALF = T // 2                 # 2048 tokens per core
PAD = 128                      # leading pad/halo columns
TP = PAD + THALF               # 2176 processed columns
HEPS = float(H) * float(np.finfo(np.float32).eps)
EPSN = 1e-5
DIL, K = 3, 4
CHUNKS = [(0, 128), (128, 512), (640, 512), (1152, 512), (1664, 512)]
F32 = mybir.dt.float32
BF = mybir.dt.float16
F32R = mybir.dt.float32r
AF = mybir.ActivationFunctionType
OP = mybir.AluOpType

_prog_cache = {}
TRACE = {"on": False, "exec_ns": None, "mean_ns": None}


def _build_program():
    nc = bacc.Bacc("TRN2", target_bir_lowering=False)

    embT = nc.declare_dram_parameter("embT", [E, TP], BF, isOutput=False)
    hsT = nc.declare_dram_parameter("hsT", [C, TP], BF, isOutput=False)
    wkT = nc.declare_dram_parameter("wkT", [E, C], BF, isOutput=False)
    wvT = nc.declare_dram_parameter("wvT", [E, H], BF, isOutput=False)
    # per-channel scalars packed into one tensor, c-tile-major columns:
    # 0:32 bk, 32:64 w12, 64:96 w12*bk, 96:104 bv, 104:232 conv_w
    cst_d = nc.declare_dram_parameter("cst", [128, 232], F32, isOutput=False)
    mask_d = nc.declare_dram_parameter("mask4", [4, TP], F32, isOutput=False)
    outT = nc.declare_dram_parameter("outT", [C, THALF], BF, isOutput=True)

    # ones-block lhsT for per-g partition reductions: col j of block g is
    # 1 if j==g (reduces a (g,h8) c-tile into row g); plus an all-ones block.
    og = np.zeros((128, G, G), np.float32)
    for g in range(G):
        og[:, g, g] = 1.0
    ones_d = nc.inline_tensor(og.reshape(128, G * G).astype(np.float32), "onesg")
    all1_d = nc.inline_tensor(np.ones((128, G), np.float32), "allones")

    rows_scr = nc.dram_tensor("rows_scr", [2 * G, TP], BF)

    with TileContext(nc) as tc:
        from contextlib import ExitStack

        with ExitStack() as ctx:
            singles = ctx.enter_context(tc.tile_pool(name="singles", bufs=1))
            # persistent small constants (single DMA + dummy ACT read so
            # downstream ACT ops don't accumulate DMA-sem waits)
            cst_t = singles.tile([128, 232], F32, tag="cst")
            mask_t = singles.tile([4, TP], F32, tag="mask")
            onesg_t = singles.tile([128, G * G], F32, tag="onesg")
            all1_t = singles.tile([128, G], F32, tag="allones")
            nc.sync.dma_start(out=cst_t, in_=cst_d[:, :])
            nc.sync.dma_start(out=mask_t, in_=mask_d[:, :])
            nc.sync.dma_start(out=onesg_t, in_=ones_d[:, :])
            nc.sync.dma_start(out=all1_t, in_=all1_d[:, :])
            dummy_t = singles.tile([128, 1], F32, tag="dummy")
            nc.scalar.copy(dummy_t, cst_t[:, 0:1])
            onesg_r = singles.tile([128, G * G], F32R, tag="onesg_r")
            nc.vector.tensor_copy(onesg_r, onesg_t)
            all1_r = singles.tile([128, G], F32R, tag="all1_r")
            nc.vector.tensor_copy(all1_r, all1_t)
            bk_t = cst_t[:, 0:32]
            w12_t = cst_t[:, 32:64]
            w12bk_t = cst_t[:, 64:96]
            bv_t = cst_t[:, 96:104]
            cw_t = cst_t[:, 104:232]

            # persistent: v^T (bias applied), bf16
            vT = [singles.tile([128, TP], BF, tag=f"vT{h8}", name=f"vT{h8}") for h8 in range(8)]
            # gate-quantity rows (bf16) + fp32 temp rows
            dot_r = singles.tile([4, TP], F32, tag="dot_r")
            ssk_r = singles.tile([4, TP], F32, tag="ssk_r")
            ssq_r = singles.tile([4, TP], F32, tag="ssq_r")
            ssv_r = singles.tile([4, TP], F32, tag="ssv_r")
            gate_r = singles.tile([4, TP], BF, tag="gate_r")
            alpha_r = singles.tile([4, TP], BF, tag="alpha_r")

            # ---------------- Stage A: matmuls + reductions ----------------
            with ExitStack() as actx:
                wpool = actx.enter_context(tc.tile_pool(name="wpool", bufs=1))
                wkT_t = [wpool.tile([128, C], BF, tag=f"wkT{e}", name=f"wkT{e}") for e in range(8)]
                wvT_t = [wpool.tile([128, H], BF, tag=f"wvT{e}", name=f"wvT{e}") for e in range(8)]
                for e in range(8):
                    nc.sync.dma_start(out=wkT_t[e], in_=wkT[e * 128:(e + 1) * 128, :])
                    nc.sync.dma_start(out=wvT_t[e], in_=wvT[e * 128:(e + 1) * 128, :])

                embP = actx.enter_context(tc.tile_pool(name="embP", bufs=10))
                sbufP = actx.enter_context(tc.tile_pool(name="sbufP", bufs=2))
                kbfP = actx.enter_context(tc.tile_pool(name="kbfP", bufs=2))
                kps_P = actx.enter_context(tc.tile_pool(name="kpsum", bufs=2, space="PSUM"))
                vps_P = actx.enter_context(tc.tile_pool(name="vpsum", bufs=2, space="PSUM"))
                red_P = actx.enter_context(tc.tile_pool(name="redpsum", bufs=1, space="PSUM"))

                for (t0, N) in CHUNKS:
                    embt = []
                    for e in range(8):
                        et = embP.tile([128, 512], BF, tag="emb", name="emb")
                        nc.sync.dma_start(out=et[:, :N], in_=embT[e * 128:(e + 1) * 128, t0:t0 + N])
                        embt.append(et)

                    dot_ps = red_P.tile([4, 512], F32, tag="dot_ps")
                    ssk_ps = red_P.tile([4, 512], F32, tag="ssk_ps")
                    ssq_ps = red_P.tile([4, 512], F32, tag="ssq_ps")
                    ssv_ps = red_P.tile([4, 512], F32, tag="ssv_ps")

                    for ct in range(32):
                        g = ct // 8
                        kps = kps_P.tile([128, 512], F32, tag="kps")
                        for e in range(8):
                            nc.tensor.matmul(
                                kps[:, :N], wkT_t[e][:, ct * 128:(ct + 1) * 128],
                                embt[e][:, :N], start=(e == 0), stop=(e == 7))
                        # key_bf = w1*w2*(key0 + bk)  (used only for the dot product)
                        kbf = kbfP.tile([128, 512], F32, tag="kbf")
                        nc.scalar.activation(
                            kbf[:, :N], kps[:, :N], AF.Identity,
                            bias=w12bk_t[:, ct:ct + 1], scale=w12_t[:, ct:ct + 1])
                        # (key0+bk)^2 for ssk
                        sq = sbufP.tile([128, 512], F32R, tag="sq")
                        nc.scalar.activation(
                            sq[:, :N], kps[:, :N], AF.Square,
                            bias=bk_t[:, ct:ct + 1], scale=1.0)
                        nc.tensor.matmul(
                            ssk_ps[:, :N], onesg_r[:, g * G:(g + 1) * G], sq[:, :N],
                            start=(ct == 0), stop=(ct == 31))
                        # hs tile
                        hst = sbufP.tile([128, 512], BF, tag="hst")
                        nc.sync.dma_start(
                            out=hst[:, :N], in_=hsT[ct * 128:(ct + 1) * 128, t0:t0 + N])
                        qq = sbufP.tile([128, 512], F32R, tag="qq")
                        nc.scalar.activation(qq[:, :N], hst[:, :N], AF.Square)
                        nc.tensor.matmul(
                            ssq_ps[:, :N], onesg_r[:, g * G:(g + 1) * G], qq[:, :N],
                            start=(ct == 0), stop=(ct == 31))
                        kq = sbufP.tile([128, 512], F32R, tag="kq")
                        nc.vector.tensor_mul(kq[:, :N], kbf[:, :N], hst[:, :N])
                        nc.tensor.matmul(
                            dot_ps[:, :N], onesg_r[:, g * G:(g + 1) * G], kq[:, :N],
                            start=(ct == 0), stop=(ct == 31))

                    for h8 in range(8):
                        vps = vps_P.tile([128, 512], F32, tag="vps")
                        for e in range(8):
                            nc.tensor.matmul(
                                vps[:, :N], wvT_t[e][:, h8 * 128:(h8 + 1) * 128],
                                embt[e][:, :N], start=(e == 0), stop=(e == 7))
                        nc.scalar.activation(
                            vT[h8][:, t0:t0 + N], vps[:, :N], AF.Identity,
                            bias=bv_t[:, h8:h8 + 1], scale=1.0)
                        vv = sbufP.tile([128, 512], F32R, tag="sq")
                        nc.scalar.activation(
                            vv[:, :N], vps[:, :N], AF.Square,
                            bias=bv_t[:, h8:h8 + 1], scale=1.0)
                        nc.tensor.matmul(
                            ssv_ps[:, :N], all1_t[:, :].bitcast(F32R), vv[:, :N].bitcast(F32R),
                            start=(h8 == 0), stop=(h8 == 7))

                    # reduction rows -> SBUF (bf16)
                    nc.scalar.activation(dot_r[:, t0:t0 + N], dot_ps[:, :N], AF.Copy)
                    nc.scalar.activation(ssk_r[:, t0:t0 + N], ssk_ps[:, :N], AF.Copy)
                    nc.scalar.activation(ssq_r[:, t0:t0 + N], ssq_ps[:, :N], AF.Copy)
                    nc.scalar.activation(ssv_r[:, t0:t0 + N], ssv_ps[:, :N], AF.Copy)

            # ---------------- gate math on [4, TP] rows ----------------
            with ExitStack() as gctx:
                rP = gctx.enter_context(tc.tile_pool(name="rowsP", bufs=1))

                def rt(tag):
                    return rP.tile([4, TP], F32, tag=tag, name="row_" + tag)

                t1 = rt("ra")
                nc.vector.tensor_scalar(t1, ssk_r, HEPS, None, op0=OP.add)
                t2 = rt("rb")
                nc.vector.tensor_scalar(t2, ssq_r, HEPS, None, op0=OP.add)
                p = rt("rc")
                nc.vector.tensor_mul(p, t1, t2)
                sp = rt("ra")
                nc.scalar.activation(sp, p, AF.Sqrt)
                rp = rt("rb")
                nc.vector.reciprocal(rp, sp)
                g1 = rt("rc")
                nc.vector.tensor_mul(g1, dot_r, rp)
                a = rt("ra")
                nc.scalar.activation(a, g1, AF.Abs, bias=0.0, scale=float(np.sqrt(H)))
                nc.vector.tensor_scalar(a, a, 1e-6, None, op0=OP.max)
                sqa = rt("rb")
                nc.scalar.activation(sqa, a, AF.Sqrt)
                sgn = rt("ra")
                nc.scalar.activation(sgn, g1, AF.Sign)
                gs = rt("rc")
                nc.vector.tensor_mul(gs, sqa, sgn)
                gate = rt("rgate")
                nc.scalar.activation(gate, gs, AF.Sigmoid)
                nc.vector.tensor_copy(gate_r[:, :], gate)
                # value rms-norm: rsn = 1/sqrt(gate^2 * ssv / H + eps_n)
                g2 = rt("ra")
                nc.vector.tensor_mul(g2, gate, gate)
                ms = rt("rb")
                nc.vector.tensor_mul(ms, g2, ssv_r)
                nc.vector.tensor_scalar(ms, ms, 1.0 / H, EPSN, op0=OP.mult, op1=OP.add)
                sm = rt("ra")
                nc.scalar.activation(sm, ms, AF.Sqrt)
                rsn = rt("rc")
                nc.vector.reciprocal(rsn, sm)
                al = rt("rb")
                nc.vector.tensor_mul(al, gate, rsn)
                nc.vector.tensor_mul(alpha_r[:, :], al, mask_t)

                nc.sync.dma_start(out=rows_scr[0:4, :], in_=gate_r[:, :])
                nc.sync.dma_start(out=rows_scr[4:8, :], in_=alpha_r[:, :])

            # ---------------- Stage B: conv + combine (channel-major) ------
            with ExitStack() as bctx:
                bcP = bctx.enter_context(tc.tile_pool(name="bcP", bufs=2))
                xnP = bctx.enter_context(tc.tile_pool(name="xnP", bufs=2))
                yP = bctx.enter_context(tc.tile_pool(name="yP", bufs=4))
                tmP = bctx.enter_context(tc.tile_pool(name="tmP", bufs=2))
                outP = bctx.enter_context(tc.tile_pool(name="outP", bufs=2))

                for g in range(G):
                    gbc = bcP.tile([128, TP], BF, tag="gbc")
                    abc = bcP.tile([128, TP], BF, tag="abc")
                    nc.gpsimd.dma_start(
                        out=gbc,
                        in_=bass.AP(tensor=rows_scr, offset=g * TP, ap=[[0, 128], [1, TP]]))
                    nc.gpsimd.dma_start(
                        out=abc,
                        in_=bass.AP(tensor=rows_scr, offset=(G + g) * TP, ap=[[0, 128], [1, TP]]))
                    for h8 in range(8):
                        ct = g * 8 + h8
                        xn = xnP.tile([128, TP], BF, tag="xn")
                        nc.vector.tensor_mul(xn, vT[h8], abc)
                        # y = sum_k w[c,k] * xn[:, PAD-9+3k + j]
                        y0 = yP.tile([128, THALF], BF, tag="y")
                        nc.vector.tensor_scalar(
                            y0, xn[:, PAD - 9:PAD - 9 + THALF],
                            cw_t[:, ct * 4:ct * 4 + 1], None, op0=OP.mult)
                        yacc = y0
                        for k in range(1, 4):
                            off = PAD - 9 + 3 * k
                            tk = tmP.tile([128, THALF], BF, tag="tk")
                            nc.vector.tensor_scalar(
                                tk, xn[:, off:off + THALF],
                                cw_t[:, ct * 4 + k:ct * 4 + k + 1], None, op0=OP.mult)
                            ynew = yP.tile([128, THALF], BF, tag="y")
                            nc.vector.tensor_add(ynew, yacc, tk)
                            yacc = ynew
                        ys = tmP.tile([128, THALF], BF, tag="ys")
                        nc.scalar.activation(ys, yacc, AF.Silu)
                        val = outP.tile([128, THALF], BF, tag="val")
                        nc.vector.tensor_mul(val, vT[h8][:, PAD:], gbc[:, PAD:])
                        ot = outP.tile([128, THALF], BF, tag="ot")
                        nc.vector.tensor_add(ot, val, ys)
                        nc.sync.dma_start(
                            out=outT[ct * 128:(ct + 1) * 128, :], in_=ot)
    nc.compile()
    return nc


def _host_prep(embeddings, hidden_states, Wv, bv, Wk, bk, w1, w2, wn, conv_w):
    """Build the 8 per-core input maps (layout/sharding prep on host)."""
    w1 = np.asarray(w1, np.float32)
    w2 = np.asarray(w2, np.float32)
    wn = np.asarray(wn, np.float32)
    w12 = (w1 * w2).reshape(C)                       # [C]
    bk_f = np.asarray(bk, np.float32).reshape(C)
    bv_f = np.asarray(bv, np.float32).reshape(H)
    # c-tile-major packs: [128, n_ctiles]
    def ctile_pack(x, ntiles):
        return np.ascontiguousarray(x.reshape(ntiles, 128).T.astype(np.float32))

    cw = np.asarray(conv_w, np.float32).reshape(C, K) * wn.reshape(C, 1)
    cst = np.concatenate([
        ctile_pack(bk_f, 32), ctile_pack(w12, 32), ctile_pack(w12 * bk_f, 32),
        ctile_pack(bv_f, 8),
        np.ascontiguousarray(
            cw.reshape(32, 128, K).transpose(1, 0, 2).reshape(128, 32 * K).astype(np.float32)),
    ], axis=1)

    wkT_b = np.ascontiguousarray(
        np.asarray(Wk, np.float32).transpose(2, 0, 1).reshape(E, C)).astype(BF16)
    wvT_b = np.ascontiguousarray(np.asarray(Wv, np.float32).T).astype(BF16)

    emb = np.asarray(embeddings, np.float32)
    hs = np.asarray(hidden_states, np.float32).reshape(B, T, C)

    in_maps = []
    for core in range(NCORES):
        b, half = core // 2, core % 2
        t0 = half * THALF
        embT_c = np.zeros((E, TP), BF16)
        hsT_c = np.zeros((C, TP), BF16)
        mask4 = np.zeros((4, TP), np.float32)
        lo = max(t0 - 9, 0)
        nh = t0 - lo                                  # halo tokens available (0 or 9)
        if nh:
            embT_c[:, PAD - nh:PAD] = emb[b, lo:t0, :].T.astype(BF16)
            hsT_c[:, PAD - nh:PAD] = hs[b, lo:t0, :].T.astype(BF16)
        embT_c[:, PAD:] = emb[b, t0:t0 + THALF, :].T.astype(BF16)
        hsT_c[:, PAD:] = hs[b, t0:t0 + THALF, :].T.astype(BF16)
        mask4[:, PAD - nh:] = 1.0
        in_maps.append({
            "embT": embT_c, "hsT": hsT_c, "wkT": wkT_b, "wvT": wvT_b,
            "cst": cst, "mask4": mask4,
        })
    return in_maps


def kernel(**inputs):
    in_maps = _host_prep(**inputs)
    if "nc" not in _prog_cache:
        _prog_cache["nc"] = _build_program()
    nc = _prog_cache["nc"]
    r = run_bass_kernel_spmd(nc, in_maps, list(range(NCORES)), trace=TRACE["on"])
    TRACE["exec_ns"] = r.exec_time_ns
    TRACE["mean_ns"] = r.mean_exec_time_ns
    res = r.results
    out = np.empty((B, T, G, H), np.float32)
    for core in range(NCORES):
        b, half = core // 2, core % 2
        oT = np.asarray(res[core]["outT"], dtype=BF16).astype(np.float32)  # [C, THALF]
        out[b, half * THALF:(half + 1) * THALF] = oT.T.reshape(THALF, G, H)
    return out


# revision 19
# speedup vs baseline: 1.1849x; 1.1849x over previous
"""Trainium2 Bass kernel for nn_EngramPt (key-gated value + dilated causal conv).

Strategy (8 cores, SPMD): shard tokens as (batch b, T-half) -> 8 shards of 2048
tokens. All device compute is channel-major ([C or H on partitions, tokens on
free dim]); host prep does the layout transposes / bf16 casts (sharding prep),
device does all the math, host re-assembles the channel-major shard outputs.

Device pipeline per core:
  key^T = WkT.T @ embT   (PE, bf16)        v^T = WvT.T @ embT  (PE)
  gate reductions (sum over H) via ones-matmuls on PE; squares on ACT;
  key*hs product on DVE.  Gate math on tiny [4, T] rows.
  x_norm^T = alpha_bcast * v^T ; 4-tap dilated conv on DVE; silu on ACT;
  out^T = gate_bcast * v^T + silu(conv)  -> bf16, host transposes back.

w1/w2 are folded into the key copy (ACT scale), wn into conv weights (host),
so non-ones norm weights are still handled exactly.
"""

import sys

if "/opt/trn_rl_repo" not in sys.path:
    sys.path.insert(0, "/opt/trn_rl_repo")

import numpy as np
import ml_dtypes

import concourse.bass as bass
import concourse.mybir as mybir
from concourse import bacc
from concourse.tile import TileContext
from concourse.bass_utils import run_bass_kernel_spmd

BF16 = np.float16

B, T, E, H, G = 4, 4096, 1024, 1024, 4
C = G * H                      # 4096 channels
NCORES = 8
THALF = T // 2                 # 2048 tokens per core
PAD = 128                      # leading pad/halo columns
TP = PAD + THALF               # 2176 processed columns
HEPS = float(H) * float(np.finfo(np.float32).eps)
EPSN = 1e-5
DIL, K = 3, 4
CHUNKS = [(0, 128), (128, 512), (640, 512), (1152, 512), (1664, 512)]
F32 = mybir.dt.float32
BF = mybir.dt.float16
F32R = mybir.dt.float32r
AF = mybir.ActivationFunctionType
OP = mybir.AluOpType

_prog_cache = {}
TRACE = {"on": False, "exec_ns": None, "mean_ns": None}


def _build_program():
    nc = bacc.Bacc("TRN2", target_bir_lowering=False)

    embT = nc.declare_dram_parameter("embT", [E, TP], BF, isOutput=False)
    hsT = nc.declare_dram_parameter("hsT", [C, TP], BF, isOutput=False)
    wkT = nc.declare_dram_parameter("wkT", [E, C], BF, isOutput=False)
    wvT = nc.declare_dram_parameter("wvT", [E, H], BF, isOutput=False)
    # per-channel scalars packed into one tensor, c-tile-major columns:
    # 0:32 bk, 32:64 w12, 64:96 w12*bk, 96:104 bv, 104:232 conv_w
    cst_d = nc.declare_dram_parameter("cst", [128, 232], F32, isOutput=False)
    mask_d = nc.declare_dram_parameter("mask4", [4, TP], BF, isOutput=False)
    outT = nc.declare_dram_parameter("outT", [C, THALF], BF, isOutput=True)

    # ones-block lhsT for per-g partition reductions: col j of block g is
    # 1 if j==g (reduces a (g,h8) c-tile into row g); plus an all-ones block.
    og = np.zeros((128, G, G), np.float32)
    for g in range(G):
        og[:, g, g] = 1.0
    ones_d = nc.inline_tensor(og.reshape(128, G * G).astype(np.float32), "onesg")
    all1_d = nc.inline_tensor(np.ones((128, G), np.float32), "allones")

    rows_scr = nc.dram_tensor("rows_scr", [2 * G, TP], BF)

    with TileContext(nc) as tc:
        from contextlib import ExitStack

        with ExitStack() as ctx:
            singles = ctx.enter_context(tc.tile_pool(name="singles", bufs=1))
            cst_t = singles.tile([128, 232], F32, tag="cst")
            mask_t = singles.tile([4, TP], BF, tag="mask")
            onesg_t = singles.tile([128, G * G], F32, tag="onesg")
            all1_t = singles.tile([128, G], F32, tag="allones")
            nc.sync.dma_start(out=cst_t, in_=cst_d[:, :])
            nc.sync.dma_start(out=mask_t, in_=mask_d[:, :])
            nc.sync.dma_start(out=onesg_t, in_=ones_d[:, :])
            nc.sync.dma_start(out=all1_t, in_=all1_d[:, :])
            dummy_t = singles.tile([128, 1], F32, tag="dummy")
            nc.scalar.copy(dummy_t, cst_t[:, 0:1])
            dummy2_t = singles.tile([4, 1], BF, tag="dummy2")
            nc.vector.tensor_copy(dummy2_t, mask_t[:, 0:1])
            onesg_r = singles.tile([128, G * G], F32R, tag="onesg_r")
            nc.vector.tensor_copy(onesg_r, onesg_t)
            all1_r = singles.tile([128, G], F32R, tag="all1_r")
            nc.vector.tensor_copy(all1_r, all1_t)
            bk_t = cst_t[:, 0:32]
            w12_t = cst_t[:, 32:64]
            w12bk_t = cst_t[:, 64:96]
            bv_t = cst_t[:, 96:104]
            cw_t = cst_t[:, 104:232]

            vT = [singles.tile([128, TP], BF, tag=f"vT{h8}", name=f"vT{h8}")
                  for h8 in range(8)]
            gate_full = singles.tile([4, TP], BF, tag="gate_full")
            al_full = singles.tile([4, TP], BF, tag="al_full")

            wpool = ctx.enter_context(tc.tile_pool(name="wpool", bufs=1))
            wkT_t = [wpool.tile([128, C], BF, tag=f"wkT{e}", name=f"wkT{e}") for e in range(8)]
            wvT_t = [wpool.tile([128, H], BF, tag=f"wvT{e}", name=f"wvT{e}") for e in range(8)]
            for e in range(8):
                nc.sync.dma_start(out=wkT_t[e], in_=wkT[e * 128:(e + 1) * 128, :])
                nc.sync.dma_start(out=wvT_t[e], in_=wvT[e * 128:(e + 1) * 128, :])

            embP = ctx.enter_context(tc.tile_pool(name="embP", bufs=8))
            sbufP = ctx.enter_context(tc.tile_pool(name="sbufP", bufs=2))
            kbfP = ctx.enter_context(tc.tile_pool(name="kbfP", bufs=2))
            rowP = ctx.enter_context(tc.tile_pool(name="rowP", bufs=1))
            bcP = ctx.enter_context(tc.tile_pool(name="bcP", bufs=2))
            xnP = ctx.enter_context(tc.tile_pool(name="xnP", bufs=2))
            yP = ctx.enter_context(tc.tile_pool(name="yP", bufs=2))
            tmP = ctx.enter_context(tc.tile_pool(name="tmP", bufs=4))
            outP = ctx.enter_context(tc.tile_pool(name="outP", bufs=2))
            kps_P = ctx.enter_context(tc.tile_pool(name="kpsum", bufs=2, space="PSUM"))
            vps_P = ctx.enter_context(tc.tile_pool(name="vpsum", bufs=2, space="PSUM"))
            red_P = ctx.enter_context(tc.tile_pool(name="redpsum", bufs=1, space="PSUM"))

            for ci, (t0, N) in enumerate(CHUNKS):
                embt = []
                for e in range(8):
                    et = embP.tile([128, 512], BF, tag="emb", name="emb")
                    nc.sync.dma_start(out=et[:, :N], in_=embT[e * 128:(e + 1) * 128, t0:t0 + N])
                    embt.append(et)

                dot_ps = red_P.tile([4, 512], F32, tag="dot_ps")
                ssk_ps = red_P.tile([4, 512], F32, tag="ssk_ps")
                ssq_ps = red_P.tile([4, 512], F32, tag="ssq_ps")
                ssv_ps = red_P.tile([4, 512], F32, tag="ssv_ps")

                for ct in range(32):
                    g = ct // 8
                    kps = kps_P.tile([128, 512], F32, tag="kps")
                    for e in range(8):
                        nc.tensor.matmul(
                            kps[:, :N], wkT_t[e][:, ct * 128:(ct + 1) * 128],
                            embt[e][:, :N], start=(e == 0), stop=(e == 7))
                    kbf = kbfP.tile([128, 512], F32, tag="kbf")
                    nc.scalar.activation(
                        kbf[:, :N], kps[:, :N], AF.Identity,
                        bias=w12bk_t[:, ct:ct + 1], scale=w12_t[:, ct:ct + 1])
                    sq = sbufP.tile([128, 512], F32R, tag="sq")
                    nc.scalar.activation(
                        sq[:, :N], kps[:, :N], AF.Square,
                        bias=bk_t[:, ct:ct + 1], scale=1.0)
                    nc.tensor.matmul(
                        ssk_ps[:, :N], onesg_r[:, g * G:(g + 1) * G], sq[:, :N],
                        start=(ct == 0), stop=(ct == 31))
                    hst = sbufP.tile([128, 512], BF, tag="hst")
                    nc.sync.dma_start(
                        out=hst[:, :N], in_=hsT[ct * 128:(ct + 1) * 128, t0:t0 + N])
                    qq = sbufP.tile([128, 512], F32R, tag="qq")
                    nc.scalar.activation(qq[:, :N], hst[:, :N], AF.Square)
                    nc.tensor.matmul(
                        ssq_ps[:, :N], onesg_r[:, g * G:(g + 1) * G], qq[:, :N],
                        start=(ct == 0), stop=(ct == 31))
                    kq = sbufP.tile([128, 512], F32R, tag="kq")
                    nc.vector.tensor_mul(kq[:, :N], kbf[:, :N], hst[:, :N])
                    nc.tensor.matmul(
                        dot_ps[:, :N], onesg_r[:, g * G:(g + 1) * G], kq[:, :N],
                        start=(ct == 0), stop=(ct == 31))

                for h8 in range(8):
                    vps = vps_P.tile([128, 512], F32, tag="vps")
                    for e in range(8):
                        nc.tensor.matmul(
                            vps[:, :N], wvT_t[e][:, h8 * 128:(h8 + 1) * 128],
                            embt[e][:, :N], start=(e == 0), stop=(e == 7))
                    nc.scalar.activation(
                        vT[h8][:, t0:t0 + N], vps[:, :N], AF.Identity,
                        bias=bv_t[:, h8:h8 + 1], scale=1.0)
                    vv = sbufP.tile([128, 512], F32R, tag="sq")
                    nc.scalar.activation(
                        vv[:, :N], vps[:, :N], AF.Square,
                        bias=bv_t[:, h8:h8 + 1], scale=1.0)
                    nc.tensor.matmul(
                        ssv_ps[:, :N], all1_r[:, :], vv[:, :N],
                        start=(h8 == 0), stop=(h8 == 7))

                # ---- per-chunk gate math on [4, N] rows ----
                def rt(tag):
                    return rowP.tile([4, 512], F32, tag=tag, name="row_" + tag)

                dot_r = rt("dot")
                nc.scalar.activation(dot_r[:, :N], dot_ps[:, :N], AF.Copy)
                ssk_r = rt("ssk")
                nc.scalar.activation(ssk_r[:, :N], ssk_ps[:, :N], AF.Copy)
                ssq_r = rt("ssq")
                nc.scalar.activation(ssq_r[:, :N], ssq_ps[:, :N], AF.Copy)
                ssv_r = rt("ssv")
                nc.scalar.activation(ssv_r[:, :N], ssv_ps[:, :N], AF.Copy)

                t1 = rt("ra")
                nc.vector.tensor_scalar(t1[:, :N], ssk_r[:, :N], HEPS, None, op0=OP.add)
                t2 = rt("rb")
                nc.vector.tensor_scalar(t2[:, :N], ssq_r[:, :N], HEPS, None, op0=OP.add)
                p = rt("rc")
                nc.vector.tensor_mul(p[:, :N], t1[:, :N], t2[:, :N])
                sp = rt("ra")
                nc.scalar.activation(sp[:, :N], p[:, :N], AF.Sqrt)
                rp = rt("rb")
                nc.vector.reciprocal(rp[:, :N], sp[:, :N])
                g1 = rt("rc")
                nc.vector.tensor_mul(g1[:, :N], dot_r[:, :N], rp[:, :N])
                a = rt("ra")
                nc.scalar.activation(a[:, :N], g1[:, :N], AF.Abs, bias=0.0,
                                     scale=float(np.sqrt(H)))
                nc.vector.tensor_scalar(a[:, :N], a[:, :N], 1e-6, None, op0=OP.max)
                sqa = rt("rb")
                nc.scalar.activation(sqa[:, :N], a[:, :N], AF.Sqrt)
                sgn = rt("ra")
                nc.scalar.activation(sgn[:, :N], g1[:, :N], AF.Sign)
                gs = rt("rc")
                nc.vector.tensor_mul(gs[:, :N], sqa[:, :N], sgn[:, :N])
                gate = rt("rgate")
                nc.scalar.activation(gate[:, :N], gs[:, :N], AF.Sigmoid)
                nc.vector.tensor_copy(gate_full[:, t0:t0 + N], gate[:, :N])
                g2 = rt("ra")
                nc.vector.tensor_mul(g2[:, :N], gate[:, :N], gate[:, :N])
                ms = rt("rb")
                nc.vector.tensor_mul(ms[:, :N], g2[:, :N], ssv_r[:, :N])
                nc.vector.tensor_scalar(ms[:, :N], ms[:, :N], 1.0 / H, EPSN,
                                        op0=OP.mult, op1=OP.add)
                sm = rt("ra")
                nc.scalar.activation(sm[:, :N], ms[:, :N], AF.Sqrt)
                rsn = rt("rc")
                nc.vector.reciprocal(rsn[:, :N], sm[:, :N])
                al = rt("rb")
                nc.vector.tensor_mul(al[:, :N], gate[:, :N], rsn[:, :N])
                alm = rt("ra")
                nc.vector.tensor_mul(alm[:, :N], al[:, :N], mask_t[:, t0:t0 + N])
                nc.vector.tensor_copy(al_full[:, t0:t0 + N], alm[:, :N])
                nc.sync.dma_start(out=rows_scr[0:4, t0:t0 + N], in_=gate_full[:, t0:t0 + N])
                nc.sync.dma_start(out=rows_scr[4:8, t0:t0 + N], in_=al_full[:, t0:t0 + N])

                # ---- stage B: 1024-wide windows, overlap next chunk's A ----
                for (w0, W) in ([(128, 512)] if ci == 1 else
                                [(640, 512)] if ci == 2 else
                                [(1152, 512)] if ci == 3 else
                                [(1664, 512)] if ci == 4 else []):
                    WX = W + 9
                    for g in range(G):
                        gbc = bcP.tile([128, 1024], BF, tag="gbc")
                        abc = bcP.tile([128, 1033], BF, tag="abc")
                        nc.gpsimd.dma_start(
                            out=gbc[:, :W],
                            in_=bass.AP(tensor=rows_scr, offset=g * TP + w0,
                                        ap=[[0, 128], [1, W]]))
                        nc.gpsimd.dma_start(
                            out=abc[:, :WX],
                            in_=bass.AP(tensor=rows_scr, offset=(G + g) * TP + w0 - 9,
                                        ap=[[0, 128], [1, WX]]))
                        for h8 in range(8):
                            ct = g * 8 + h8
                            xn = xnP.tile([128, 1033], BF, tag="xn")
                            nc.vector.tensor_mul(
                                xn[:, :WX], vT[h8][:, w0 - 9:w0 + W], abc[:, :WX])
                            y0 = yP.tile([128, 1024], BF, tag="y")
                            nc.vector.tensor_scalar(
                                y0[:, :W], xn[:, 0:W],
                                cw_t[:, ct * 4:ct * 4 + 1], None, op0=OP.mult)
                            yacc = y0
                            for k in range(1, 4):
                                tk = tmP.tile([128, 1024], BF, tag="tm")
                                nc.vector.tensor_scalar(
                                    tk[:, :W], xn[:, 3 * k:3 * k + W],
                                    cw_t[:, ct * 4 + k:ct * 4 + k + 1], None, op0=OP.mult)
                                ynew = yP.tile([128, 1024], BF, tag="y")
                                nc.vector.tensor_add(ynew[:, :W], yacc[:, :W], tk[:, :W])
                                yacc = ynew
                            ys = tmP.tile([128, 1024], BF, tag="tm")
                            nc.scalar.activation(ys[:, :W], yacc[:, :W], AF.Silu)
                            val = tmP.tile([128, 1024], BF, tag="tm")
                            nc.vector.tensor_mul(
                                val[:, :W], vT[h8][:, w0:w0 + W], gbc[:, :W])
                            ot = outP.tile([128, 1024], BF, tag="ot")
                            nc.vector.tensor_add(ot[:, :W], val[:, :W], ys[:, :W])
                            nc.sync.dma_start(
                                out=outT[ct * 128:(ct + 1) * 128, w0 - PAD:w0 - PAD + W],
                                in_=ot[:, :W])
    nc.compile()
    return nc


def _host_prep(embeddings, hidden_states, Wv, bv, Wk, bk, w1, w2, wn, conv_w):
    """Build the 8 per-core input maps (layout/sharding prep on host)."""
    w1 = np.asarray(w1, np.float32)
    w2 = np.asarray(w2, np.float32)
    wn = np.asarray(wn, np.float32)
    w12 = (w1 * w2).reshape(C)                       # [C]
    bk_f = np.asarray(bk, np.float32).reshape(C)
    bv_f = np.asarray(bv, np.float32).reshape(H)
    # c-tile-major packs: [128, n_ctiles]
    def ctile_pack(x, ntiles):
        return np.ascontiguousarray(x.reshape(ntiles, 128).T.astype(np.float32))

    cw = np.asarray(conv_w, np.float32).reshape(C, K) * wn.reshape(C, 1)
    cst = np.concatenate([
        ctile_pack(bk_f, 32), ctile_pack(w12, 32), ctile_pack(w12 * bk_f, 32),
        ctile_pack(bv_f, 8),
        np.ascontiguousarray(
            cw.reshape(32, 128, K).transpose(1, 0, 2).reshape(128, 32 * K).astype(np.float32)),
    ], axis=1)

    wkT_b = np.ascontiguousarray(
        np.asarray(Wk, np.float32).transpose(2, 0, 1).reshape(E, C)).astype(BF16)
    wvT_b = np.ascontiguousarray(np.asarray(Wv, np.float32).T).astype(BF16)

    emb = np.asarray(embeddings, np.float32)
    hs = np.asarray(hidden_states, np.float32).reshape(B, T, C)

    in_maps = []
    for core in range(NCORES):
        b, half = core // 2, core % 2
        t0 = half * THALF
        embT_c = np.zeros((E, TP), BF16)
        hsT_c = np.zeros((C, TP), BF16)
        mask4 = np.zeros((4, TP), np.float16)
        lo = max(t0 - 9, 0)
        nh = t0 - lo                                  # halo tokens available (0 or 9)
        if nh:
            embT_c[:, PAD - nh:PAD] = emb[b, lo:t0, :].T.astype(BF16)
            hsT_c[:, PAD - nh:PAD] = hs[b, lo:t0, :].T.astype(BF16)
        embT_c[:, PAD:] = emb[b, t0:t0 + THALF, :].T.astype(BF16)
        hsT_c[:, PAD:] = hs[b, t0:t0 + THALF, :].T.astype(BF16)
        mask4[:, PAD - nh:] = 1.0
        in_maps.append({
            "embT": embT_c, "hsT": hsT_c, "wkT": wkT_b, "wvT": wvT_b,
            "cst": cst, "mask4": mask4,
        })
    return in_maps


def kernel(**inputs):
    in_maps = _host_prep(**inputs)
    if "nc" not in _prog_cache:
        _prog_cache["nc"] = _build_program()
    nc = _prog_cache["nc"]
    r = run_bass_kernel_spmd(nc, in_maps, list(range(NCORES)), trace=TRACE["on"])
    TRACE["exec_ns"] = r.exec_time_ns
    TRACE["mean_ns"] = r.mean_exec_time_ns
    res = r.results
    out = np.empty((B, T, G, H), np.float32)
    for core in range(NCORES):
        b, half = core // 2, core % 2
        oT = np.asarray(res[core]["outT"], dtype=BF16).astype(np.float32)  # [C, THALF]
        out[b, half * THALF:(half + 1) * THALF] = oT.T.reshape(THALF, G, H)
    return out


# revision 20
# speedup vs baseline: 1.2106x; 1.0217x over previous
"""Trainium2 Bass kernel for nn_EngramPt (key-gated value + dilated causal conv).

Strategy (8 cores, SPMD): shard tokens as (batch b, T-half) -> 8 shards of 2048
tokens. All device compute is channel-major ([C or H on partitions, tokens on
free dim]); host prep does the layout transposes / bf16 casts (sharding prep),
device does all the math, host re-assembles the channel-major shard outputs.

Device pipeline per core:
  key^T = WkT.T @ embT   (PE, bf16)        v^T = WvT.T @ embT  (PE)
  gate reductions (sum over H) via ones-matmuls on PE; squares on ACT;
  key*hs product on DVE.  Gate math on tiny [4, T] rows.
  x_norm^T = alpha_bcast * v^T ; 4-tap dilated conv on DVE; silu on ACT;
  out^T = gate_bcast * v^T + silu(conv)  -> bf16, host transposes back.

w1/w2 are folded into the key copy (ACT scale), wn into conv weights (host),
so non-ones norm weights are still handled exactly.
"""

import sys

if "/opt/trn_rl_repo" not in sys.path:
    sys.path.insert(0, "/opt/trn_rl_repo")

import numpy as np
import ml_dtypes

import concourse.bass as bass
import concourse.mybir as mybir
from concourse import bacc
from concourse.tile import TileContext
from concourse.bass_utils import run_bass_kernel_spmd

BF16 = np.float16

B, T, E, H, G = 4, 4096, 1024, 1024, 4
C = G * H                      # 4096 channels
NCORES = 8
THALF = T // 2                 # 2048 tokens per core
PAD = 128                      # leading pad/halo columns
TP = PAD + THALF               # 2176 processed columns
HEPS = float(H) * float(np.finfo(np.float32).eps)
EPSN = 1e-5
DIL, K = 3, 4
CHUNKS = [(0, 128), (128, 512), (640, 512), (1152, 512), (1664, 512)]
F32 = mybir.dt.float32
BF = mybir.dt.float16
F32R = mybir.dt.float32r
AF = mybir.ActivationFunctionType
OP = mybir.AluOpType

_prog_cache = {}
TRACE = {"on": False, "exec_ns": None, "mean_ns": None}


def _build_program():
    nc = bacc.Bacc("TRN2", target_bir_lowering=False)

    embT = nc.declare_dram_parameter("embT", [E, TP], BF, isOutput=False)
    hsT = nc.declare_dram_parameter("hsT", [C, TP], BF, isOutput=False)
    wkT = nc.declare_dram_parameter("wkT", [E, C], BF, isOutput=False)
    wvT = nc.declare_dram_parameter("wvT", [E, H], BF, isOutput=False)
    # per-channel scalars packed into one tensor, c-tile-major columns:
    # 0:32 bk, 32:64 w12, 64:96 w12*bk, 96:104 bv, 104:232 conv_w
    cst_d = nc.declare_dram_parameter("cst", [128, 232], F32, isOutput=False)
    mask_d = nc.declare_dram_parameter("mask4", [4, TP], BF, isOutput=False)
    outT = nc.declare_dram_parameter("outT", [C, THALF], BF, isOutput=True)

    # ones-block lhsT for per-g partition reductions: col j of block g is
    # 1 if j==g (reduces a (g,h8) c-tile into row g); plus an all-ones block.
    og = np.zeros((128, G, G), np.float32)
    for g in range(G):
        og[:, g, g] = 1.0
    ones_d = nc.inline_tensor(og.reshape(128, G * G).astype(np.float32), "onesg")
    all1_d = nc.inline_tensor(np.ones((128, G), np.float32), "allones")

    rows_scr = nc.dram_tensor("rows_scr", [2 * G, TP], BF)

    with TileContext(nc) as tc:
        from contextlib import ExitStack

        with ExitStack() as ctx:
            singles = ctx.enter_context(tc.tile_pool(name="singles", bufs=1))
            cst_t = singles.tile([128, 232], F32, tag="cst")
            mask_t = singles.tile([4, TP], BF, tag="mask")
            onesg_t = singles.tile([128, G * G], F32, tag="onesg")
            all1_t = singles.tile([128, G], F32, tag="allones")
            nc.sync.dma_start(out=cst_t, in_=cst_d[:, :])
            nc.sync.dma_start(out=mask_t, in_=mask_d[:, :])
            nc.sync.dma_start(out=onesg_t, in_=ones_d[:, :])
            nc.sync.dma_start(out=all1_t, in_=all1_d[:, :])
            dummy_t = singles.tile([128, 1], F32, tag="dummy")
            nc.scalar.copy(dummy_t, cst_t[:, 0:1])
            dummy2_t = singles.tile([4, 1], BF, tag="dummy2")
            nc.vector.tensor_copy(dummy2_t, mask_t[:, 0:1])
            onesg_r = singles.tile([128, G * G], F32R, tag="onesg_r")
            nc.vector.tensor_copy(onesg_r, onesg_t)
            all1_r = singles.tile([128, G], F32R, tag="all1_r")
            nc.vector.tensor_copy(all1_r, all1_t)
            bk_t = cst_t[:, 0:32]
            w12_t = cst_t[:, 32:64]
            w12bk_t = cst_t[:, 64:96]
            bv_t = cst_t[:, 96:104]
            cw_t = cst_t[:, 104:232]

            vT = [singles.tile([128, TP], BF, tag=f"vT{h8}", name=f"vT{h8}")
                  for h8 in range(8)]
            gate_full = singles.tile([4, TP], BF, tag="gate_full")
            al_full = singles.tile([4, TP], BF, tag="al_full")

            wpool = ctx.enter_context(tc.tile_pool(name="wpool", bufs=1))
            wkT_t = [wpool.tile([128, C], BF, tag=f"wkT{e}", name=f"wkT{e}") for e in range(8)]
            wvT_t = [wpool.tile([128, H], BF, tag=f"wvT{e}", name=f"wvT{e}") for e in range(8)]
            for e in range(8):
                nc.sync.dma_start(out=wkT_t[e], in_=wkT[e * 128:(e + 1) * 128, :])
                nc.sync.dma_start(out=wvT_t[e], in_=wvT[e * 128:(e + 1) * 128, :])

            embP = ctx.enter_context(tc.tile_pool(name="embP", bufs=8))
            sbufP = ctx.enter_context(tc.tile_pool(name="sbufP", bufs=3))
            kbfP = ctx.enter_context(tc.tile_pool(name="kbfP", bufs=3))
            rowP = ctx.enter_context(tc.tile_pool(name="rowP", bufs=1))
            bcP = ctx.enter_context(tc.tile_pool(name="bcP", bufs=2))
            xnP = ctx.enter_context(tc.tile_pool(name="xnP", bufs=2))
            yP = ctx.enter_context(tc.tile_pool(name="yP", bufs=2))
            tmP = ctx.enter_context(tc.tile_pool(name="tmP", bufs=4))
            outP = ctx.enter_context(tc.tile_pool(name="outP", bufs=2))
            kps_P = ctx.enter_context(tc.tile_pool(name="kpsum", bufs=2, space="PSUM"))
            vps_P = ctx.enter_context(tc.tile_pool(name="vpsum", bufs=2, space="PSUM"))
            red_P = ctx.enter_context(tc.tile_pool(name="redpsum", bufs=1, space="PSUM"))

            for ci, (t0, N) in enumerate(CHUNKS):
                embt = []
                for e in range(8):
                    et = embP.tile([128, 512], BF, tag="emb", name="emb")
                    nc.sync.dma_start(out=et[:, :N], in_=embT[e * 128:(e + 1) * 128, t0:t0 + N])
                    embt.append(et)

                dot_ps = red_P.tile([4, 512], F32, tag="dot_ps")
                ssk_ps = red_P.tile([4, 512], F32, tag="ssk_ps")
                ssq_ps = red_P.tile([4, 512], F32, tag="ssq_ps")
                ssv_ps = red_P.tile([4, 512], F32, tag="ssv_ps")

                for ct in range(32):
                    g = ct // 8
                    kps = kps_P.tile([128, 512], F32, tag="kps")
                    for e in range(8):
                        nc.tensor.matmul(
                            kps[:, :N], wkT_t[e][:, ct * 128:(ct + 1) * 128],
                            embt[e][:, :N], start=(e == 0), stop=(e == 7))
                    kbf = kbfP.tile([128, 512], F32, tag="kbf")
                    nc.scalar.activation(
                        kbf[:, :N], kps[:, :N], AF.Identity,
                        bias=w12bk_t[:, ct:ct + 1], scale=w12_t[:, ct:ct + 1])
                    sq = sbufP.tile([128, 512], F32R, tag="sq")
                    nc.scalar.activation(
                        sq[:, :N], kps[:, :N], AF.Square,
                        bias=bk_t[:, ct:ct + 1], scale=1.0)
                    nc.tensor.matmul(
                        ssk_ps[:, :N], onesg_r[:, g * G:(g + 1) * G], sq[:, :N],
                        start=(ct == 0), stop=(ct == 31))
                    hst = sbufP.tile([128, 512], BF, tag="hst")
                    nc.sync.dma_start(
                        out=hst[:, :N], in_=hsT[ct * 128:(ct + 1) * 128, t0:t0 + N])
                    qq = sbufP.tile([128, 512], F32R, tag="qq")
                    nc.scalar.activation(qq[:, :N], hst[:, :N], AF.Square)
                    nc.tensor.matmul(
                        ssq_ps[:, :N], onesg_r[:, g * G:(g + 1) * G], qq[:, :N],
                        start=(ct == 0), stop=(ct == 31))
                    kq = sbufP.tile([128, 512], F32R, tag="kq")
                    nc.vector.tensor_mul(kq[:, :N], kbf[:, :N], hst[:, :N])
                    nc.tensor.matmul(
                        dot_ps[:, :N], onesg_r[:, g * G:(g + 1) * G], kq[:, :N],
                        start=(ct == 0), stop=(ct == 31))

                for h8 in range(8):
                    vps = vps_P.tile([128, 512], F32, tag="vps")
                    for e in range(8):
                        nc.tensor.matmul(
                            vps[:, :N], wvT_t[e][:, h8 * 128:(h8 + 1) * 128],
                            embt[e][:, :N], start=(e == 0), stop=(e == 7))
                    nc.scalar.activation(
                        vT[h8][:, t0:t0 + N], vps[:, :N], AF.Identity,
                        bias=bv_t[:, h8:h8 + 1], scale=1.0)
                    vv = sbufP.tile([128, 512], F32R, tag="sq")
                    nc.scalar.activation(
                        vv[:, :N], vps[:, :N], AF.Square,
                        bias=bv_t[:, h8:h8 + 1], scale=1.0)
                    nc.tensor.matmul(
                        ssv_ps[:, :N], all1_r[:, :], vv[:, :N],
                        start=(h8 == 0), stop=(h8 == 7))

                # ---- per-chunk gate math on [4, N] rows ----
                def rt(tag):
                    return rowP.tile([4, 512], F32, tag=tag, name="row_" + tag)

                dot_r = rt("dot")
                nc.scalar.activation(dot_r[:, :N], dot_ps[:, :N], AF.Copy)
                ssk_r = rt("ssk")
                nc.scalar.activation(ssk_r[:, :N], ssk_ps[:, :N], AF.Copy)
                ssq_r = rt("ssq")
                nc.scalar.activation(ssq_r[:, :N], ssq_ps[:, :N], AF.Copy)
                ssv_r = rt("ssv")
                nc.scalar.activation(ssv_r[:, :N], ssv_ps[:, :N], AF.Copy)

                t1 = rt("ra")
                nc.vector.tensor_scalar(t1[:, :N], ssk_r[:, :N], HEPS, None, op0=OP.add)
                t2 = rt("rb")
                nc.vector.tensor_scalar(t2[:, :N], ssq_r[:, :N], HEPS, None, op0=OP.add)
                p = rt("rc")
                nc.vector.tensor_mul(p[:, :N], t1[:, :N], t2[:, :N])
                sp = rt("ra")
                nc.scalar.activation(sp[:, :N], p[:, :N], AF.Sqrt)
                rp = rt("rb")
                nc.vector.reciprocal(rp[:, :N], sp[:, :N])
                g1 = rt("rc")
                nc.vector.tensor_mul(g1[:, :N], dot_r[:, :N], rp[:, :N])
                a = rt("ra")
                nc.scalar.activation(a[:, :N], g1[:, :N], AF.Abs, bias=0.0,
                                     scale=float(np.sqrt(H)))
                nc.vector.tensor_scalar(a[:, :N], a[:, :N], 1e-6, None, op0=OP.max)
                sqa = rt("rb")
                nc.scalar.activation(sqa[:, :N], a[:, :N], AF.Sqrt)
                sgn = rt("ra")
                nc.scalar.activation(sgn[:, :N], g1[:, :N], AF.Sign)
                gs = rt("rc")
                nc.vector.tensor_mul(gs[:, :N], sqa[:, :N], sgn[:, :N])
                gate = rt("rgate")
                nc.scalar.activation(gate[:, :N], gs[:, :N], AF.Sigmoid)
                nc.vector.tensor_copy(gate_full[:, t0:t0 + N], gate[:, :N])
                g2 = rt("ra")
                nc.vector.tensor_mul(g2[:, :N], gate[:, :N], gate[:, :N])
                ms = rt("rb")
                nc.vector.tensor_mul(ms[:, :N], g2[:, :N], ssv_r[:, :N])
                nc.vector.tensor_scalar(ms[:, :N], ms[:, :N], 1.0 / H, EPSN,
                                        op0=OP.mult, op1=OP.add)
                sm = rt("ra")
                nc.scalar.activation(sm[:, :N], ms[:, :N], AF.Sqrt)
                rsn = rt("rc")
                nc.vector.reciprocal(rsn[:, :N], sm[:, :N])
                al = rt("rb")
                nc.vector.tensor_mul(al[:, :N], gate[:, :N], rsn[:, :N])
                alm = rt("ra")
                nc.vector.tensor_mul(alm[:, :N], al[:, :N], mask_t[:, t0:t0 + N])
                nc.vector.tensor_copy(al_full[:, t0:t0 + N], alm[:, :N])
                nc.sync.dma_start(out=rows_scr[0:4, t0:t0 + N], in_=gate_full[:, t0:t0 + N])
                nc.sync.dma_start(out=rows_scr[4:8, t0:t0 + N], in_=al_full[:, t0:t0 + N])

                # ---- stage B: 1024-wide windows, overlap next chunk's A ----
                for (w0, W) in ([(128, 512)] if ci == 1 else
                                [(640, 512)] if ci == 2 else
                                [(1152, 512)] if ci == 3 else
                                [(1664, 512)] if ci == 4 else []):
                    WX = W + 9
                    for g in range(G):
                        gbc = bcP.tile([128, 1024], BF, tag="gbc")
                        abc = bcP.tile([128, 1033], BF, tag="abc")
                        nc.gpsimd.dma_start(
                            out=gbc[:, :W],
                            in_=bass.AP(tensor=rows_scr, offset=g * TP + w0,
                                        ap=[[0, 128], [1, W]]))
                        nc.gpsimd.dma_start(
                            out=abc[:, :WX],
                            in_=bass.AP(tensor=rows_scr, offset=(G + g) * TP + w0 - 9,
                                        ap=[[0, 128], [1, WX]]))
                        for h8 in range(8):
                            ct = g * 8 + h8
                            xn = xnP.tile([128, 1033], BF, tag="xn")
                            nc.vector.tensor_mul(
                                xn[:, :WX], vT[h8][:, w0 - 9:w0 + W], abc[:, :WX])
                            y0 = yP.tile([128, 1024], BF, tag="y")
                            nc.vector.tensor_scalar(
                                y0[:, :W], xn[:, 0:W],
                                cw_t[:, ct * 4:ct * 4 + 1], None, op0=OP.mult)
                            yacc = y0
                            for k in range(1, 4):
                                tk = tmP.tile([128, 1024], BF, tag="tm")
                                nc.vector.tensor_scalar(
                                    tk[:, :W], xn[:, 3 * k:3 * k + W],
                                    cw_t[:, ct * 4 + k:ct * 4 + k + 1], None, op0=OP.mult)
                                ynew = yP.tile([128, 1024], BF, tag="y")
                                nc.vector.tensor_add(ynew[:, :W], yacc[:, :W], tk[:, :W])
                                yacc = ynew
                            ys = tmP.tile([128, 1024], BF, tag="tm")
                            nc.scalar.activation(ys[:, :W], yacc[:, :W], AF.Silu)
                            val = tmP.tile([128, 1024], BF, tag="tm")
                            nc.vector.tensor_mul(
                                val[:, :W], vT[h8][:, w0:w0 + W], gbc[:, :W])
                            ot = outP.tile([128, 1024], BF, tag="ot")
                            nc.vector.tensor_add(ot[:, :W], val[:, :W], ys[:, :W])
                            nc.sync.dma_start(
                                out=outT[ct * 128:(ct + 1) * 128, w0 - PAD:w0 - PAD + W],
                                in_=ot[:, :W])
    nc.compile()
    return nc


def _host_prep(embeddings, hidden_states, Wv, bv, Wk, bk, w1, w2, wn, conv_w):
    """Build the 8 per-core input maps (layout/sharding prep on host)."""
    w1 = np.asarray(w1, np.float32)
    w2 = np.asarray(w2, np.float32)
    wn = np.asarray(wn, np.float32)
    w12 = (w1 * w2).reshape(C)                       # [C]
    bk_f = np.asarray(bk, np.float32).reshape(C)
    bv_f = np.asarray(bv, np.float32).reshape(H)
    # c-tile-major packs: [128, n_ctiles]
    def ctile_pack(x, ntiles):
        return np.ascontiguousarray(x.reshape(ntiles, 128).T.astype(np.float32))

    cw = np.asarray(conv_w, np.float32).reshape(C, K) * wn.reshape(C, 1)
    cst = np.concatenate([
        ctile_pack(bk_f, 32), ctile_pack(w12, 32), ctile_pack(w12 * bk_f, 32),
        ctile_pack(bv_f, 8),
        np.ascontiguousarray(
            cw.reshape(32, 128, K).transpose(1, 0, 2).reshape(128, 32 * K).astype(np.float32)),
    ], axis=1)

    wkT_b = np.ascontiguousarray(
        np.asarray(Wk, np.float32).transpose(2, 0, 1).reshape(E, C)).astype(BF16)
    wvT_b = np.ascontiguousarray(np.asarray(Wv, np.float32).T).astype(BF16)

    emb = np.asarray(embeddings, np.float32)
    hs = np.asarray(hidden_states, np.float32).reshape(B, T, C)

    in_maps = []
    for core in range(NCORES):
        b, half = core // 2, core % 2
        t0 = half * THALF
        embT_c = np.zeros((E, TP), BF16)
        hsT_c = np.zeros((C, TP), BF16)
        mask4 = np.zeros((4, TP), np.float16)
        lo = max(t0 - 9, 0)
        nh = t0 - lo                                  # halo tokens available (0 or 9)
        if nh:
            embT_c[:, PAD - nh:PAD] = emb[b, lo:t0, :].T.astype(BF16)
            hsT_c[:, PAD - nh:PAD] = hs[b, lo:t0, :].T.astype(BF16)
        embT_c[:, PAD:] = emb[b, t0:t0 + THALF, :].T.astype(BF16)
        hsT_c[:, PAD:] = hs[b, t0:t0 + THALF, :].T.astype(BF16)
        mask4[:, PAD - nh:] = 1.0
        in_maps.append({
            "embT": embT_c, "hsT": hsT_c, "wkT": wkT_b, "wvT": wvT_b,
            "cst": cst, "mask4": mask4,
        })
    return in_maps


def kernel(**inputs):
    in_maps = _host_prep(**inputs)
    if "nc" not in _prog_cache:
        _prog_cache["nc"] = _build_program()
    nc = _prog_cache["nc"]
    r = run_bass_kernel_spmd(nc, in_maps, list(range(NCORES)), trace=TRACE["on"])
    TRACE["exec_ns"] = r.exec_time_ns
    TRACE["mean_ns"] = r.mean_exec_time_ns
    res = r.results
    out = np.empty((B, T, G, H), np.float32)
    for core in range(NCORES):
        b, half = core // 2, core % 2
        oT = np.asarray(res[core]["outT"], dtype=BF16).astype(np.float32)  # [C, THALF]
        out[b, half * THALF:(half + 1) * THALF] = oT.T.reshape(THALF, G, H)
    return out
